# revision 17
# baseline (speedup 1.0000x reference)
"""Trainium2 Bass kernel for nn_CrossAttention (3-head cross-attention + ReLU projection).

Sharding: data-parallel over batch. B=8 -> one batch element per NeuronCore,
identical SPMD program, no collectives. Full inputs in, full output out.

Per-core dataflow (v2 - transposed-context architecture):
  t1,t2 [1024,768] --PE transpose (batched psum eviction)--> t1T,t2T [768,1024]
  per head h:
    qT = Wq_h^T-stationary matmuls over t1T (+bq fused in ACT eviction)  [768,1024]
    kT = same from t2T (+bk)                                             [768,1024]
    V  = t2T-stationary matmuls with Wv_h (+bv fused in DVE evict, bf16) [1024,768]
    per q-tile (128 rows):
      S = qT^T @ kT (PSUM, both 512-halves interleaved, one stationary/e)
      P = Exp(S - 20) in bf16 (2 ACT halves, accum_out -> rowsum halves)
      PT chunks via DMA-transpose (xbar): P [128,1024] -> PT[k%128, kc, q]
    per q-half (512 q):
      ctxT[e,q] = sum_kc V[kc,e]^T-stationary @ PT[:,kc,qhalf]   (bf16 matmuls)
      mT = Relu(ctxT) bf16 (ACT eviction; softmax normalization deferred:
           relu(x/r) = relu(x)/r for r>0, and projection is linear in x)
      out_acc[qt] (+)= (mT^T-stationary @ Wp_h) * (1/rowsum) via DVE
           scalar_tensor_tensor; bp added via in1 on head 0
  out = out_acc after head 2 (per-qtile DMA)
"""
import numpy as np

import concourse.bass as bass
import concourse.mybir as mybir
from concourse import bacc
from concourse.tile import TileContext

F32 = mybir.dt.float32
F32R = mybir.dt.float32r
BF16 = mybir.dt.bfloat16
AF = mybir.ActivationFunctionType
AX = mybir.AxisListType
ALU = mybir.AluOpType

L = 1024          # LQ = LK
H = 768           # H1 = H2
NH = 3            # heads
HC = H // 128     # 6 chunks of the hidden dim
LC = L // 128     # 8 chunks of the seq dim
QT = L // 128     # 8 q-tiles
DEBUG = False
EXP_BIAS = -20.0  # constant shift inside Exp; safe: |score| <~ 55 (std 9.2)

_CACHE = {}


def build():
    nc = bacc.Bacc()
    t1 = nc.declare_dram_parameter("t1", [L, H], F32R, isOutput=False)
    t2 = nc.declare_dram_parameter("t2", [L, H], F32R, isOutput=False)
    wq = nc.declare_dram_parameter("wq", [NH, H, H], F32R, isOutput=False)
    wk = nc.declare_dram_parameter("wk", [NH, H, H], F32R, isOutput=False)
    wv = nc.declare_dram_parameter("wv", [NH, H, H], F32R, isOutput=False)
    wp_bf = nc.declare_dram_parameter("wp_bf", [NH * H, H], BF16, isOutput=False)
    bq_sb = nc.declare_dram_parameter("bq_sb", [NH, 128, HC], F32, isOutput=False)
    bk_sb = nc.declare_dram_parameter("bk_sb", [NH, 128, HC], F32, isOutput=False)
    bv_bc = nc.declare_dram_parameter("bv_bc", [NH, 128, H], F32, isOutput=False)
    bp_bc = nc.declare_dram_parameter("bp_bc", [128, H], F32, isOutput=False)
    ident_d = nc.declare_dram_parameter("ident", [128, 128], F32R, isOutput=False)
    out_d = nc.declare_dram_parameter("out", [L, H], F32, isOutput=True)
    if DEBUG:
        dbg_pt = nc.declare_dram_parameter("dbg_pt", [128, LC, 256], BF16, isOutput=True)
        dbg_mt = nc.declare_dram_parameter("dbg_mt", [128, HC, 256], BF16, isOutput=True)
        dbg_ri = nc.declare_dram_parameter("dbg_ri", [128, QT], F32, isOutput=True)
        dbg_oa = nc.declare_dram_parameter("dbg_oa", [128, H], F32, isOutput=True)

    with TileContext(nc) as tc:
        with tc.tile_pool(name="psA", bufs=2, space="PSUM") as psA, \
             tc.tile_pool(name="psB", bufs=2, space="PSUM") as psB, \
             tc.tile_pool(name="psT", bufs=2, space="PSUM") as psT, \
             tc.tile_pool(name="pers", bufs=1) as pers, \
             tc.tile_pool(name="natp", bufs=4) as natp, \
             tc.tile_pool(name="work", bufs=2) as work, \
             tc.tile_pool(name="ptp", bufs=2) as ptp, \
             tc.tile_pool(name="mtp", bufs=2) as mtp, \
             tc.tile_pool(name="wqk", bufs=9) as wqk, \
             tc.tile_pool(name="wpp", bufs=1) as wpp, \
             tc.tile_pool(name="hb", bufs=1) as hb, \
             tc.tile_pool(name="stats", bufs=4) as stats:

            ident = pers.tile([128, 128], F32R, name="ident")
            nc.sync.dma_start(out=ident[:], in_=ident_d[:])
            identb = pers.tile([128, 128], BF16, name="identb")
            nc.scalar.activation(identb[:], ident[:], AF.Copy, bias=0.0, scale=1.0)
            nbias = pers.tile([128, 1], F32, name="nbias")
            nc.vector.memset(nbias[:], EXP_BIAS)

            t1T = pers.tile([128, HC * L], F32R, name="t1T")
            t2T = pers.tile([128, HC * L], F32R, name="t2T")
            qTt = pers.tile([128, HC * L], F32R, name="qTt")
            kTt = pers.tile([128, HC * L], F32R, name="kTt")
            Vt = pers.tile([128, LC * H], BF16, name="Vt")
            oa = [pers.tile([128, H], F32, name=f"oa{i}") for i in range(QT)]
            rinvs = [pers.tile([128, QT], F32, name=f"rinv{h}") for h in range(NH)]

            def transpose_in(srcd, dstT):
                # 2 groups of 4 seq-chunks; per d, 4 transposes batched into
                # one [128,512] psum tile -> single DVE eviction
                for cg in range(LC // 4):
                    nats = []
                    for j in range(4):
                        c = cg * 4 + j
                        nat = natp.tile([128, H], F32R, name="nat", tag="nat")
                        nc.sync.dma_start(out=nat[:], in_=srcd[c * 128:(c + 1) * 128, :])
                        nats.append(nat)
                    for d in range(HC):
                        pt = psT.tile([128, 512], F32R, tag="tr")
                        for j in range(4):
                            nc.tensor.transpose(
                                pt[:, j * 128:(j + 1) * 128],
                                nats[j][:, d * 128:(d + 1) * 128], ident[:])
                        nc.vector.tensor_copy(
                            dstT[:, d * L + cg * 512: d * L + (cg + 1) * 512], pt[:])

            def load_w(wsrc, h):
                ws = []
                for d in range(HC):
                    wt = wqk.tile([128, H], F32R, name="w", tag="w")
                    nc.sync.dma_start(out=wt[:], in_=wsrc[h, d * 128:(d + 1) * 128, :])
                    ws.append(wt)
                return ws

            def load_wk_split(h):
                # wk chunks 0-2 through the weight ring, 3-5 through the nat
                # pool (idle outside phase 0) so all six prefetch during the
                # previous head's attention instead of stalling k-proj
                ws = []
                for d in range(3):
                    wt = wqk.tile([128, H], F32R, name="w", tag="w")
                    nc.sync.dma_start(out=wt[:], in_=wk[h, d * 128:(d + 1) * 128, :])
                    ws.append(wt)
                for d in range(3, HC):
                    wt = natp.tile([128, H], F32R, name="nat", tag="nat")
                    nc.sync.dma_start(out=wt[:], in_=wk[h, d * 128:(d + 1) * 128, :])
                    ws.append(wt)
                return ws

            def proj_qk(wch, srcT, dstT, bias):
                for e in range(HC):
                    for qh in range(2):
                        ps = psB.tile([128, 512], F32, tag="b")
                        for d in range(HC):
                            nc.tensor.matmul(
                                ps[:],
                                wch[d][:, e * 128:(e + 1) * 128],
                                srcT[:, d * L + qh * 512: d * L + (qh + 1) * 512],
                                start=(d == 0), stop=(d == HC - 1))
                        nc.scalar.activation(
                            dstT[:, e * L + qh * 512: e * L + (qh + 1) * 512],
                            ps[:], AF.Identity, bias=bias[:, e:e + 1], scale=1.0)

            def proj_v(wch, bvb):
                for kc in range(LC):
                    for (n0, nw) in ((0, 512), (512, 256)):
                        ps = psB.tile([128, nw], F32, tag="b")
                        for d in range(HC):
                            nc.tensor.matmul(
                                ps[:],
                                t2T[:, d * L + kc * 128: d * L + (kc + 1) * 128],
                                wch[d][:, n0:n0 + nw],
                                start=(d == 0), stop=(d == HC - 1))
                        nc.vector.tensor_add(
                            Vt[:, kc * H + n0: kc * H + n0 + nw],
                            ps[:], bvb[:, n0:n0 + nw])

            def attention(h, wpt):
                rinv = rinvs[h]
                PThs = [None] * 4
                mThs = [None] * 6
                Ps = {}

                def scores_exp(qt):
                    sA = psA.tile([128, 1024], F32, tag="s")
                    for e in range(HC):
                        lhs = qTt[:, e * L + qt * 128: e * L + (qt + 1) * 128]
                        for nh2 in range(2):
                            nc.tensor.matmul(
                                sA[:, nh2 * 512:(nh2 + 1) * 512],
                                lhs,
                                kTt[:, e * L + nh2 * 512: e * L + (nh2 + 1) * 512],
                                start=(e == 0), stop=(e == HC - 1))
                    P = work.tile([128, 1024], BF16, name="P", tag="P")
                    es0 = stats.tile([128, 1], F32, tag="e0")
                    es1 = stats.tile([128, 1], F32, tag="e1")
                    nc.scalar.activation(P[:, 0:512], sA[:, 0:512], AF.Exp,
                                         bias=nbias[:], scale=1.0, accum_out=es0[:])
                    nc.scalar.activation(P[:, 512:1024], sA[:, 512:1024], AF.Exp,
                                         bias=nbias[:], scale=1.0, accum_out=es1[:])
                    esum = stats.tile([128, 1], F32, tag="es")
                    nc.vector.tensor_add(esum[:], es0[:], es1[:])
                    nc.vector.reciprocal(rinv[:, qt:qt + 1], esum[:])
                    Ps[qt] = P

                def ptrans(qt):
                    # PE transposes of P(qt); emitted one q-tile behind the
                    # scores stream so the ACT Exp overlaps scores(qt+1)
                    P = Ps.pop(qt)
                    g = qt // 2
                    if qt % 2 == 0:
                        PThs[g] = ptp.tile([128, LC, 256], BF16, name="PTh", tag="pt")
                    pp = psT.tile([128, LC * 128], BF16, tag="tr")
                    for kc in range(LC):
                        nc.tensor.transpose(pp[:, kc * 128:(kc + 1) * 128],
                                            P[:, kc * 128:(kc + 1) * 128], identb[:])
                    nc.vector.tensor_copy(
                        PThs[g][:, :, (qt % 2) * 128:(qt % 2 + 1) * 128],
                        pp[:].rearrange("p (c j) -> p c j", j=128))

                def ctx_quarter(g):
                    mTh = mtp.tile([128, HC, 256], BF16, name="mTh", tag="mt")
                    mThs[g] = mTh
                    for et in range(HC):
                        ps = psB.tile([128, 256], F32, tag="b")
                        for kc in range(LC):
                            nc.tensor.matmul(
                                ps[:],
                                Vt[:, kc * H + et * 128: kc * H + (et + 1) * 128],
                                PThs[g][:, kc, :],
                                start=(kc == 0), stop=(kc == LC - 1))
                        nc.scalar.activation(mTh[:, et, :], ps[:], AF.Relu,
                                             bias=0.0, scale=1.0)

                def ctx_qtile(qt):
                    # last-quarter split: ctx for one q-tile as soon as its
                    # own PT columns land, so the tail chain pipelines
                    mTh = mtp.tile([128, HC, 128], BF16, name="mTq", tag="mt")
                    mThs[4 + (qt % 2)] = mTh
                    col = (qt % 2) * 128
                    for et in range(HC):
                        ps = psB.tile([128, 128], F32, tag="b")
                        for kc in range(LC):
                            nc.tensor.matmul(
                                ps[:],
                                Vt[:, kc * H + et * 128: kc * H + (et + 1) * 128],
                                PThs[3][:, kc, col:col + 128],
                                start=(kc == 0), stop=(kc == LC - 1))
                        nc.scalar.activation(mTh[:, et, :], ps[:], AF.Relu,
                                             bias=0.0, scale=1.0)

                def outproj_qtile(qt):
                    mTh = mThs[4 + (qt % 2)]
                    for (n0, nw) in ((0, 512), (512, 256)):
                        ps = psB.tile([128, nw], F32, tag="b")
                        for et in range(HC):
                            nc.tensor.matmul(
                                ps[:],
                                mTh[:, et, :],
                                wpt[:, et * H + n0: et * H + n0 + nw],
                                start=(et == 0), stop=(et == HC - 1))
                        dst = oa[qt][:, n0:n0 + nw]
                        acc = bp_t[:, n0:n0 + nw] if h == 0 else dst
                        nc.vector.scalar_tensor_tensor(
                            dst, ps[:], rinv[:, qt:qt + 1], acc,
                            op0=ALU.mult, op1=ALU.add)
                        if h == NH - 1:
                            nc.sync.dma_start(
                                out=out_d[qt * 128:(qt + 1) * 128, n0:n0 + nw],
                                in_=oa[qt][:, n0:n0 + nw])

                def outproj_quarter(g):
                    for ql in range(2):
                        qt = g * 2 + ql
                        for (n0, nw) in ((0, 512), (512, 256)):
                            ps = psB.tile([128, nw], F32, tag="b")
                            for et in range(HC):
                                nc.tensor.matmul(
                                    ps[:],
                                    mThs[g][:, et, ql * 128:(ql + 1) * 128],
                                    wpt[:, et * H + n0: et * H + n0 + nw],
                                    start=(et == 0), stop=(et == HC - 1))
                            dst = oa[qt][:, n0:n0 + nw]
                            acc = bp_t[:, n0:n0 + nw] if h == 0 else dst
                            nc.vector.scalar_tensor_tensor(
                                dst, ps[:], rinv[:, qt:qt + 1], acc,
                                op0=ALU.mult, op1=ALU.add)
                        if h == NH - 1:
                            nc.sync.dma_start(
                                out=out_d[qt * 128:(qt + 1) * 128, :], in_=oa[qt][:])

                # software pipeline: ctx(g) runs one q-tile after its PT pair
                # is in flight; outproj(g) one segment after ctx(g)
                scores_exp(0)
                scores_exp(1)
                ptrans(0)
                scores_exp(2)
                ptrans(1)
                ctx_quarter(0)
                scores_exp(3)
                ptrans(2)
                outproj_quarter(0)
                if DEBUG and h == 0:
                    nc.sync.dma_start(out=dbg_pt[:], in_=PThs[0][:])
                    nc.sync.dma_start(out=dbg_mt[:], in_=mThs[0][:])
                    nc.sync.dma_start(out=dbg_oa[:], in_=oa[0][:])
                scores_exp(4)
                ptrans(3)
                ctx_quarter(1)
                scores_exp(5)
                ptrans(4)
                outproj_quarter(1)
                scores_exp(6)
                ptrans(5)
                ctx_quarter(2)
                scores_exp(7)
                ptrans(6)
                outproj_quarter(2)
                ctx_qtile(6)
                ptrans(7)
                outproj_qtile(6)
                ctx_qtile(7)
                outproj_qtile(7)
                if DEBUG and h == 0:
                    nc.sync.dma_start(out=dbg_ri[:], in_=rinv[:])

            # ---------------- phase 0: inputs + head-0 projections ----------
            transpose_in(t1, t1T)
            bqs = hb.tile([128, HC], F32, name="bqs", tag="bqs", bufs=2)
            nc.sync.dma_start(out=bqs[:], in_=bq_sb[0])
            wch = load_w(wq, 0)
            bp_t = pers.tile([128, H], F32, name="bp_t")
            nc.sync.dma_start(out=bp_t[:], in_=bp_bc[:])
            proj_qk(wch, t1T, qTt, bqs)
            transpose_in(t2, t2T)
            bks = hb.tile([128, HC], F32, name="bks", tag="bks", bufs=2)
            nc.sync.dma_start(out=bks[:], in_=bk_sb[0])
            wch = load_w(wk, 0)
            proj_qk(wch, t2T, kTt, bks)
            bvb = hb.tile([128, H], F32, name="bvb", tag="bvb", bufs=1)
            nc.sync.dma_start(out=bvb[:], in_=bv_bc[0])
            wch = load_w(wv, 0)
            proj_v(wch, bvb)

            # ---------------- head loop ----------------
            nxt = None
            for h in range(NH):
                wpt = wpp.tile([128, HC * H], BF16, name="wpt", tag="wpt")
                for c in range(HC):
                    nc.sync.dma_start(
                        out=wpt[:, c * H:(c + 1) * H],
                        in_=wp_bf[(h * HC + c) * 128:(h * HC + c + 1) * 128, :])
                if h + 1 < NH:
                    # prefetch next head's weights during this head's attention
                    bqs = hb.tile([128, HC], F32, name="bqs", tag="bqs", bufs=2)
                    nc.sync.dma_start(out=bqs[:], in_=bq_sb[h + 1])
                    bks = hb.tile([128, HC], F32, name="bks", tag="bks", bufs=2)
                    nc.sync.dma_start(out=bks[:], in_=bk_sb[h + 1])
                    bvb = hb.tile([128, H], F32, name="bvb", tag="bvb", bufs=1)
                    nc.sync.dma_start(out=bvb[:], in_=bv_bc[h + 1])
                    nxt = (load_w(wq, h + 1), load_w(wk, h + 1), load_w(wv, h + 1),
                           bqs, bks, bvb)
                attention(h, wpt)
                if h + 1 < NH:
                    wqn, wkn, wvn, bqs, bks, bvb = nxt
                    proj_qk(wqn, t1T, qTt, bqs)
                    proj_qk(wkn, t2T, kTt, bks)
                    proj_v(wvn, bvb)

    nc.finalize()
    return nc


def kernel(tensor1, tensor2, Wq, bq, Wk, bk, Wv, bv, Wp, bp):
    import ml_dtypes
    from concourse.bass_utils import run_bass_kernel_spmd

    B = tensor1.shape[0]
    assert B == 8
    if "nc" not in _CACHE:
        _CACHE["nc"] = build()
    nc = _CACHE["nc"]

    f32 = np.float32
    shared = {
        "wq": np.ascontiguousarray(Wq, dtype=f32),
        "wk": np.ascontiguousarray(Wk, dtype=f32),
        "wv": np.ascontiguousarray(Wv, dtype=f32),
        "wp_bf": np.ascontiguousarray(np.asarray(Wp, dtype=f32)).astype(ml_dtypes.bfloat16),
        "bq_sb": np.ascontiguousarray(
            bq.reshape(NH, HC, 128).transpose(0, 2, 1), dtype=f32),
        "bk_sb": np.ascontiguousarray(
            bk.reshape(NH, HC, 128).transpose(0, 2, 1), dtype=f32),
        "bv_bc": np.ascontiguousarray(
            np.broadcast_to(np.asarray(bv, dtype=f32)[:, None, :], (NH, 128, H))),
        "bp_bc": np.ascontiguousarray(
            np.broadcast_to(np.asarray(bp, dtype=f32)[None, :], (128, H))),
        "ident": np.eye(128, dtype=f32),
    }
    in_maps = [
        dict(shared,
             t1=np.ascontiguousarray(tensor1[b], dtype=f32),
             t2=np.ascontiguousarray(tensor2[b], dtype=f32))
        for b in range(B)
    ]
    res = run_bass_kernel_spmd(nc, in_maps, list(range(B)))
    return np.stack([res.results[b]["out"] for b in range(B)], axis=0)
